# revision 11
# baseline (speedup 1.0000x reference)
"""Trainium2 Bass kernel for BilinearInteractionV2.

out[b, p, e] = (sum_d femb[b, left[p], d] * W[p, d, e]) * femb[b, right[p], e]

feature_emb: [2048, 40, 64] f32, bilinear_W: [780, 64, 64] f32,
left/right idx = upper-triangle pairs in combinations order (left-major).

Sharding: batch split across 8 NeuronCores (pure data parallel), W replicated.

Per-core plan (B_local = 256 = 2 b-tiles of 128 rows), HW-measured:
output is computed and stored as bf16 (upcast to f32 on the host; the
rel-err budget absorbs the ~2e-3 rounding), so the steady-state HBM
stream is 25.6MB stores + 6.4MB W ~= 89us at the ~358GB/s HBM/NC cap and
the compute chain (~130us with sem latencies) is what binds:
  - Output stores get the sync HWDGE ring to themselves; W and femb loads
    go through the gpsimd SWDGE ring (sharing the store ring serializes
    with stores; issuing loads from the Act queue stalls behind drains).
  - W host-swizzled to [d, p, e] and cast bf16: halves the replicated-W
    read (12.8->6.4MB) and makes each partition's DMA line contiguous.
    femb b-tile resident in SBUF f32; per-field PE transposes (+copy to
    bf16 femT) put d on partitions for the matmul stationary operand.
  - Pairs grouped by left field (combinations order is left-major); per
    unit of <=16 pairs one stationary femT field slice serves 2 chunked
    bf16 matmuls [64,128]^T @ [64, 8*64] -> PSUM [128, 16*64]. Small
    units with a 3-deep PSUM pool pipeline the PE->mul handoff (deeper
    beats bigger once the compute chain is the bottleneck).
  - The elementwise *right multiply (PSUM f32 operand, so no 2x DVE
    mode) would bottleneck the DVE alone (~120us busy); ~1/3 of units
    (the first of each flush + every 6th) instead drain PSUM->stage on
    the Activation engine and multiply in place on the (PSUM-portless)
    GpSimd engine. gpsimd-first keeps the slower gpsimd mul off the
    flush's critical tail.
  - Stage tiles flush to DRAM in ~1.5MB bf16 stores per supergroup (~96
    pairs, smaller head groups for fast fill, small tail for drain).
"""

import os
import time

import numpy as np

import concourse.bass as bass
import concourse.mybir as mybir
import concourse.tile as tile
from concourse import bacc
from concourse.bass_utils import run_bass_kernel_spmd
from concourse.masks import make_identity

N_CORES = 8
BATCH = 2048
B_LOCAL = BATCH // N_CORES  # 256
B_TILES = B_LOCAL // 128    # 2
NF = 40
D = 64
PAIRS = NF * (NF - 1) // 2  # 780
F32 = mybir.dt.float32
F32R = mybir.dt.float32r
BF16 = mybir.dt.bfloat16

# W (and the matmul lhsT) in bf16: halves the dominant replicated-W HBM
# read (12.8MB -> 6.4MB per core). Precision: bf16 product terms over a
# 64-term contraction give ~2e-3 relative error, far inside the 2e-2 gate.
W_BF16 = int(os.environ.get("K_W_BF16", "1"))
W_DT = BF16 if W_BF16 else F32R
# femb bf16: 1 = bf16 end-to-end — halves the femb HBM read AND makes the
# C-mode (Act-drain + DVE 2x bf16 mul) stage multiply all-bf16, unlocking
# the DVE 2x_1p perf mode; 2 = load bf16 and upcast once to f32 on Act.
FEMB_BF16 = int(os.environ.get("K_FEMB_BF16", "1"))
FEMB_DT = BF16 if FEMB_BF16 == 1 else F32
FEMB_DRAM_DT = BF16 if FEMB_BF16 else F32
# Store the output as bf16 and upcast to f32 on the host: halves the
# dominant 51.1MB/core store stream (the harness gates on rel err 2e-2;
# bf16 output rounding adds ~2e-3). The returned array is still f32.
OUT_BF16 = int(os.environ.get("K_OUT_BF16", "1"))
OUT_DT = BF16 if OUT_BF16 else F32

# pairs per matmul: 8 -> N = 512 cols = one PSUM bank; 16 -> one 1024-col
# matmul spanning 2 banks (HW-verified correct), halving PE instruction
# count (the PE sequencer is near-saturated dispatching ldweights+matmul).
CHUNK_PAIRS = int(os.environ.get("K_CHUNK_PAIRS", "8"))
MUL_CHUNKS = int(os.environ.get("K_MUL_CHUNKS", str(16 // CHUNK_PAIRS)))
SG_TARGET_PAIRS = int(os.environ.get("K_SG", "96"))
STAGE_BUFS = int(os.environ.get("K_STAGE_BUFS", "4"))
W_BUFS = int(os.environ.get("K_W_BUFS", "3"))
MM_BUFS = int(os.environ.get("K_MM_BUFS", "3"))
MERGE_FLUSH = int(os.environ.get("K_MERGE_FLUSH", "0"))
FEMT_POOL = int(os.environ.get("K_FEMT_POOL", "0"))

# The elementwise multiply is the compute bottleneck if one engine does it
# all. Each <=16-pair unit is assigned one of three modes:
#   A: DVE multiplies straight from PSUM (f32 operand -> 1x DVE mode)
#   B: Act drains PSUM->bf16 stage; Pool (gpsimd) multiplies in place
#   C: Act drains PSUM->bf16 stage; DVE multiplies in place in bf16,
#      which hits the DVE 2x_1p perf mode (all operands 2-byte packed)
# Weighted round-robin across units balances DVE/Act/Pool busy time at
# ~65-70us each, under the ~100us HBM floor. Weights via K_WA/K_WB/K_WC.
W_A = int(os.environ.get("K_WA", "5"))
W_B = int(os.environ.get("K_WB", "4"))
W_C = int(os.environ.get("K_WC", "5"))
# 1: route the FIRST unit of each flush to B (Pool's slower mul starts
# earliest so the store is tail-gated by the faster DVE muls instead).
B_FIRST = int(os.environ.get("K_B_FIRST", "1"))
FEMB_ENG = os.environ.get("K_FEMB_ENG", "gpsimd")
W_ENG = os.environ.get("K_W_ENG", "gpsimd")
FEMT_ENG = os.environ.get("K_FEMT_ENG", "scalar")
# 1: transposes stage through the matmul PSUM pool instead of a dedicated
# 2-bank psT pool, freeing those banks for a 4th matmul buffer.
PST_SHARE = int(os.environ.get("K_PST_SHARE", "0"))
# 1: host supplies femb pre-transposed [f, d, b] bf16; one DMA loads all
# femT, eliminating the PE-transpose + Act-copy chain entirely.
FEMT_DMA = int(os.environ.get("K_FEMT_DMA", "1"))
# Head-load chunking: fields [0, F0) of femb (rhs operand; sg0 reads right
# fields 1..16) and [0, F0T) of femT (matmul lhsT; early sgs use left
# fields 0..3) load first on the gpsimd queue ahead of W; the remaining
# fields stream on TAIL_ENG's queue so they neither gate the first flush
# nor delay the sg1+ W loads behind them.
F0 = int(os.environ.get("K_F0", "20"))
F0T = int(os.environ.get("K_F0T", "6"))
TAIL_ENG = os.environ.get("K_TAIL_ENG", "scalar")


def _mul_units():
    """Units of <=MUL_CHUNKS*CHUNK_PAIRS pairs within one left-field group.

    Yields (left_field, pair_start, [chunk_cnts], right_field_start)."""
    p0 = 0
    for l in range(NF - 1):
        k = NF - 1 - l
        for u0 in range(0, k, MUL_CHUNKS * CHUNK_PAIRS):
            npair = min(MUL_CHUNKS * CHUNK_PAIRS, k - u0)
            cnts = []
            left = npair
            while left > 0:
                cnts.append(min(CHUNK_PAIRS, left))
                left -= cnts[-1]
            yield (l, p0 + u0, cnts, l + 1 + u0)
        p0 += k


def _sg_size_plan():
    """Supergroup pair-count targets: small head (fast pipeline fill) and
    small tail (fast drain), big middle (DMA efficiency)."""
    hmode = int(os.environ.get("K_HEAD16", "1"))
    head = {0: [32, 64], 1: [16, 32, 64], 2: [8, 16, 32, 64]}[hmode]
    tail = [48, 32, 16]
    mid = PAIRS - sum(head) - sum(tail)
    n_mid = max(1, round(mid / SG_TARGET_PAIRS + 0.5))
    base = mid // n_mid
    plan = head + [base + (1 if i < mid - base * n_mid else 0)
                   for i in range(n_mid)] + tail
    assert sum(plan) == PAIRS
    return plan


def _supergroups():
    """Group consecutive mul-units into flush units per the size plan.

    Each supergroup closes once it REACHES its target (so it may overshoot
    by up to one unit); this avoids a cascade of tiny spill groups."""
    plan = _sg_size_plan()
    sgs, cur, cnt, pi = [], [], 0, 0
    for u in _mul_units():
        cur.append(u)
        cnt += sum(u[2])
        if cnt >= plan[min(pi, len(plan) - 1)]:
            sgs.append(cur)
            cur, cnt = [], 0
            pi += 1
    if cur:
        sgs.append(cur)
    return sgs


def _sg_max_pairs():
    return max(sum(sum(u[2]) for u in sg) for sg in _supergroups())


def _body(nc, tc, pools, femb, w, out, mode="full"):
    do_load = mode in ("full", "load", "compute")
    do_compute = mode in ("full", "compute")
    do_store = mode in ("full", "store")
    femb_pool, femT_pool, ident_pool, w_pool, psT_pool, psum_pool, stage_pool = pools

    femb, femb_T = femb
    femb_eng = getattr(nc, FEMB_ENG)
    w_eng = getattr(nc, W_ENG)
    femT_eng = getattr(nc, FEMT_ENG)

    if not FEMT_DMA:
        ident = ident_pool.tile([128, 128], FEMB_DT)
        make_identity(nc, ident)

    # Per-b-tile resident tiles: femb rows + transposed fields. The field
    # transposes are emitted just-in-time (per supergroup) so the first
    # flush is not gated on all 80 of them.
    femb_t = []
    femT = []

    def _load_femb(eng, bt, ft, f_lo, f_hi):
        eng.dma_start(
            out=ft[:, f_lo * D:f_hi * D],
            in_=femb[bt * 128:(bt + 1) * 128, f_lo:f_hi].rearrange(
                "b f d -> b (f d)"),
        )

    def _load_femT(eng, bt, tt, f_lo, f_hi):
        # [f, d, b] -> partitions=d, per-field 128-col b slices
        eng.dma_start(
            out=tt[:, f_lo * 128:f_hi * 128].rearrange(
                "d (f b) -> d f b", f=f_hi - f_lo),
            in_=femb_T[f_lo:f_hi, :, bt * 128:(bt + 1) * 128].rearrange(
                "f d b -> d f b"),
        )

    f_head = min(F0, NF)
    fT_head = min(F0T, NF)
    for bt in range(B_TILES):
        ft = femb_pool.tile([128, NF * D], FEMB_DT, tag="femb")
        if do_load:
            if FEMB_BF16 == 2:
                raw = femb_pool.tile([128, NF * D], BF16, tag="fraw")
                femb_eng.dma_start(
                    out=raw,
                    in_=femb[bt * 128:(bt + 1) * 128].rearrange(
                        "b f d -> b (f d)"),
                )
                nc.scalar.copy(ft, raw)
            else:
                _load_femb(femb_eng, bt, ft, 0, f_head)
        femb_t.append(ft)
        if not FEMT_POOL:
            tt = femT_pool.tile([64, NF * 128], W_DT, tag="femT")
            if FEMT_DMA and do_load:
                _load_femT(nc.gpsimd, bt, tt, 0, fT_head)
            femT.append(tt)

    def _emit_tail_loads():
        if not do_load:
            return
        tail_eng = getattr(nc, TAIL_ENG)
        for bt in range(B_TILES):
            if FEMB_BF16 != 2 and f_head < NF:
                _load_femb(tail_eng, bt, femb_t[bt], f_head, NF)
            if FEMT_DMA and not FEMT_POOL and fT_head < NF:
                _load_femT(tail_eng, bt, femT[bt], fT_head, NF)

    transposed = [dict() for _ in range(B_TILES)]

    def ensure_fields(bt, fields):
        if FEMT_DMA:
            return
        for f in fields:
            if f in transposed[bt]:
                continue
            ps = (psum_pool if PST_SHARE else psT_pool).tile(
                [64, 128], FEMB_DT, tag="mm" if PST_SHARE else None)
            nc.tensor.transpose(ps, femb_t[bt][:, f * D:(f + 1) * D], ident)

            def _cp(dst, src):
                if FEMT_ENG == "vector":
                    nc.vector.tensor_copy(dst, src)
                else:
                    femT_eng.copy(dst, src)

            if FEMT_POOL:
                ftile = femT_pool.tile([64, 128], W_DT, tag="femT")
                _cp(ftile, ps)
                transposed[bt][f] = ftile
            else:
                _cp(femT[bt][:, f * 128:(f + 1) * 128], ps)
                transposed[bt][f] = True

    def lhsT_for(bt, f):
        if FEMT_POOL:
            return transposed[bt][f]
        return femT[bt][:, f * 128:(f + 1) * 128]

    # Smooth weighted round-robin over unit modes; forced-B units (first of
    # each flush) bill against B's share so the global ratio stays on target.
    wsum = {"A": W_A, "B": W_B, "C": W_C}
    credit = {m: 0.0 for m in "ABC"}
    tot_w = sum(wsum.values())

    def pick_mode(force_b):
        for m in "ABC":
            credit[m] += wsum[m]
        if force_b and W_B > 0:
            m = "B"
        else:
            m = max("ABC", key=lambda k: (credit[k], k))
        credit[m] -= tot_w
        return m

    first_sg = True
    for sg in _supergroups():
        sg_p0 = sg[0][1]
        sg_np = sum(sum(u[2]) for u in sg)
        # One DMA loads this supergroup's W; both b-tiles reuse it.
        wsg = w_pool.tile([64, _sg_max_pairs(), D], W_DT, tag="w")
        if do_load:
            w_eng.dma_start(
                out=wsg[:, :sg_np, :],
                in_=w[:, sg_p0:sg_p0 + sg_np, :] if W_BF16
                else w[:, sg_p0:sg_p0 + sg_np, :].bitcast(F32R),
            )
        if first_sg:
            _emit_tail_loads()
            first_sg = False
        if MERGE_FLUSH:
            mstage = stage_pool.tile([128, B_TILES, _sg_max_pairs() * D],
                                     OUT_DT, tag="stage")
        for bt in range(B_TILES):
            if do_compute:
                ensure_fields(bt, sorted({u[0] for u in sg}))
            if MERGE_FLUSH:
                stage = mstage[:, bt, :]
            else:
                stage = stage_pool.tile([128, _sg_max_pairs() * D], OUT_DT,
                                        tag="stage")
            if do_store and not do_compute:
                # touch the tile so Tile allocates it; garbage data is fine
                nc.vector.memset(stage[:, :8], 0.0)
            if do_compute:
                for ui, (l, p0, cnts, r0) in enumerate(sg):
                    un = sum(cnts)
                    ps = psum_pool.tile([128, MUL_CHUNKS * CHUNK_PAIRS * D], F32,
                                        tag="mm")
                    off = 0
                    for cnt in cnts:
                        woff = p0 - sg_p0 + off
                        nc.tensor.matmul(
                            ps[:, off * D:(off + cnt) * D],
                            lhsT=lhsT_for(bt, l),
                            rhs=wsg[:, woff:woff + cnt, :].rearrange("d p e -> d (p e)"),
                            start=True,
                            stop=True,
                        )
                        off += cnt
                    co = (p0 - sg_p0) * D
                    st = stage[:, co:co + un * D]
                    rhs_fe = femb_t[bt][:, r0 * D:(r0 + un) * D]
                    m = pick_mode(B_FIRST and ui == 0)
                    if m == "A":
                        nc.vector.tensor_mul(st, ps[:, :un * D], rhs_fe)
                    elif m == "B":
                        # Act drains PSUM->stage; Pool multiplies in place.
                        nc.scalar.copy(st, ps[:, :un * D])
                        nc.gpsimd.tensor_mul(st, st, rhs_fe)
                    else:
                        # Act drains PSUM->stage; DVE 2x bf16 mul in place.
                        nc.scalar.copy(st, ps[:, :un * D])
                        nc.vector.tensor_mul(st, st, rhs_fe)
            if do_store and not MERGE_FLUSH:
                nc.sync.dma_start(
                    out=out[bt * 128:(bt + 1) * 128, sg_p0:sg_p0 + sg_np].rearrange(
                        "b p e -> b (p e)"
                    ),
                    in_=stage[:, :sg_np * D],
                )
        if do_store and MERGE_FLUSH:
            nc.sync.dma_start(
                out=out[:, sg_p0:sg_p0 + sg_np].rearrange(
                    "(t b) p e -> b t (p e)", b=128
                ),
                in_=mstage[:, :, :sg_np * D],
            )


def build_kernel(reps: int = 1, mode: str = "full") -> bass.Bass:
    """Build + finalize the per-core Bass module.

    reps > 1 wraps the whole body in a hardware loop (for timing runs)."""
    nc = bacc.Bacc("TRN2", target_bir_lowering=False)
    femb = nc.dram_tensor("feature_emb", [B_LOCAL, NF, D], FEMB_DRAM_DT,
                          kind="ExternalInput")
    femb_T = (nc.dram_tensor("femb_T", [NF, D, B_LOCAL], BF16,
                             kind="ExternalInput") if FEMT_DMA else None)
    # host-swizzled to [d, p, e] for contiguous per-partition DMA lines
    w = nc.dram_tensor("bilinear_W", [D, PAIRS, D], BF16 if W_BF16 else F32,
                       kind="ExternalInput")
    out = nc.dram_tensor("out", [B_LOCAL, PAIRS, D], OUT_DT,
                         kind="ExternalOutput")

    with tile.TileContext(nc) as tc:
        from contextlib import ExitStack
        with ExitStack() as _stack:
            femb_pool = _stack.enter_context(
                tc.tile_pool(name="femb", bufs=B_TILES))
            femT_pool = _stack.enter_context(
                tc.tile_pool(name="femT", bufs=(12 if FEMT_POOL else B_TILES)))
            ident_pool = _stack.enter_context(tc.tile_pool(name="ident", bufs=1))
            w_pool = _stack.enter_context(tc.tile_pool(name="w", bufs=W_BUFS))
            psum_pool = _stack.enter_context(
                tc.tile_pool(name="mm", bufs=MM_BUFS, space="PSUM"))
            psT_pool = psum_pool if PST_SHARE else _stack.enter_context(
                tc.tile_pool(name="psT", bufs=2, space="PSUM"))
            stage_pool = _stack.enter_context(
                tc.tile_pool(name="stage", bufs=STAGE_BUFS))
            pools = (femb_pool, femT_pool, ident_pool, w_pool, psT_pool,
                     psum_pool, stage_pool)
            if reps == 1:
                _body(nc, tc, pools, (femb, femb_T), w, out, mode)
            else:
                with tc.For_i(0, reps, 1):
                    _body(nc, tc, pools, (femb, femb_T), w, out, mode)
    nc.finalize()
    return nc


_CACHED_NC = None


def make_in_maps(feature_emb: np.ndarray, bilinear_W: np.ndarray):
    feature_emb = np.ascontiguousarray(np.asarray(feature_emb, dtype=np.float32))
    if FEMB_BF16:
        import ml_dtypes
        feature_emb = feature_emb.astype(ml_dtypes.bfloat16)
    bilinear_W = np.asarray(bilinear_W, dtype=np.float32)
    assert feature_emb.shape == (BATCH, NF, D)
    assert bilinear_W.shape == (PAIRS, D, D)
    w_swz = np.ascontiguousarray(bilinear_W.transpose(1, 0, 2))  # [d, p, e]
    if W_BF16:
        import ml_dtypes
        w_swz = w_swz.astype(ml_dtypes.bfloat16)
    maps = []
    for c in range(N_CORES):
        m = {
            "feature_emb": feature_emb[c * B_LOCAL:(c + 1) * B_LOCAL],
            "bilinear_W": w_swz,
        }
        if FEMT_DMA:
            import ml_dtypes
            fT = np.asarray(
                m["feature_emb"], dtype=np.float32).transpose(1, 2, 0)
            m["femb_T"] = np.ascontiguousarray(fT.astype(ml_dtypes.bfloat16))
        maps.append(m)
    return maps


def kernel(feature_emb: np.ndarray, bilinear_W: np.ndarray,
           left_idx: np.ndarray = None, right_idx: np.ndarray = None,
           **_ignored) -> np.ndarray:
    global _CACHED_NC
    if _CACHED_NC is None:
        _CACHED_NC = build_kernel(reps=1)
    nc = _CACHED_NC

    in_maps = make_in_maps(feature_emb, bilinear_W)
    # The NRT occasionally reports a transient "exec unit unrecoverable" if a
    # previous process wedged a core; it clears on retry.
    last_err = None
    for attempt in range(3):
        try:
            res = run_bass_kernel_spmd(nc, in_maps, list(range(N_CORES)))
            break
        except Exception as e:  # noqa: BLE001
            last_err = e
            time.sleep(5.0)
    else:
        raise last_err
    full = np.concatenate([res.results[c]["out"] for c in range(N_CORES)],
                          axis=0)
    return np.ascontiguousarray(full.astype(np.float32))



# revision 23
# speedup vs baseline: 1.1979x; 1.1979x over previous
"""Trainium2 Bass kernel for BilinearInteractionV2.

out[b, p, e] = (sum_d femb[b, left[p], d] * W[p, d, e]) * femb[b, right[p], e]

feature_emb: [2048, 40, 64] f32, bilinear_W: [780, 64, 64] f32,
left/right idx = upper-triangle pairs in combinations order (left-major).

Sharding: batch split across 8 NeuronCores (pure data parallel), W replicated.

Per-core plan (B_local = 256 = 2 b-tiles of 128 rows), HW-measured:
output is computed and stored as bf16 (upcast to f32 on the host; the
rel-err budget absorbs the ~2e-3 rounding), so the steady-state HBM
stream is 25.6MB stores + 6.4MB W ~= 89us at the ~358GB/s HBM/NC cap and
the compute chain (~130us with sem latencies) is what binds:
  - Output stores get the sync HWDGE ring to themselves; W and femb loads
    go through the gpsimd SWDGE ring (sharing the store ring serializes
    with stores; issuing loads from the Act queue stalls behind drains).
  - W host-swizzled to [d, p, e] and cast bf16: halves the replicated-W
    read (12.8->6.4MB) and makes each partition's DMA line contiguous.
    femb b-tile resident in SBUF f32; per-field PE transposes (+copy to
    bf16 femT) put d on partitions for the matmul stationary operand.
  - Pairs grouped by left field (combinations order is left-major); per
    unit of <=16 pairs one stationary femT field slice serves 2 chunked
    bf16 matmuls [64,128]^T @ [64, 8*64] -> PSUM [128, 16*64]. Small
    units with a 3-deep PSUM pool pipeline the PE->mul handoff (deeper
    beats bigger once the compute chain is the bottleneck).
  - The elementwise *right multiply (PSUM f32 operand, so no 2x DVE
    mode) would bottleneck the DVE alone (~120us busy); ~1/3 of units
    (the first of each flush + every 6th) instead drain PSUM->stage on
    the Activation engine and multiply in place on the (PSUM-portless)
    GpSimd engine. gpsimd-first keeps the slower gpsimd mul off the
    flush's critical tail.
  - Stage tiles flush to DRAM in ~1.5MB bf16 stores per supergroup (~96
    pairs, smaller head groups for fast fill, small tail for drain).
"""

import os
import time

import numpy as np

import concourse.bass as bass
import concourse.mybir as mybir
import concourse.tile as tile
from concourse import bacc
from concourse.bass_utils import run_bass_kernel_spmd
from concourse.masks import make_identity

N_CORES = 8
BATCH = 2048
B_LOCAL = BATCH // N_CORES  # 256
B_TILES = B_LOCAL // 128    # 2
NF = 40
D = 64
PAIRS = NF * (NF - 1) // 2  # 780
F32 = mybir.dt.float32
F32R = mybir.dt.float32r
BF16 = mybir.dt.bfloat16

# W (and the matmul lhsT) in bf16: halves the dominant replicated-W HBM
# read (12.8MB -> 6.4MB per core). Precision: bf16 product terms over a
# 64-term contraction give ~2e-3 relative error, far inside the 2e-2 gate.
W_BF16 = int(os.environ.get("K_W_BF16", "1"))
W_DT = BF16 if W_BF16 else F32R
# femb bf16: 1 = bf16 end-to-end — halves the femb HBM read AND makes the
# C-mode (Act-drain + DVE 2x bf16 mul) stage multiply all-bf16, unlocking
# the DVE 2x_1p perf mode; 2 = load bf16 and upcast once to f32 on Act.
FEMB_BF16 = int(os.environ.get("K_FEMB_BF16", "1"))
FEMB_DT = BF16 if FEMB_BF16 == 1 else F32
FEMB_DRAM_DT = BF16 if FEMB_BF16 else F32
# Store the output as bf16 and upcast to f32 on the host: halves the
# dominant 51.1MB/core store stream (the harness gates on rel err 2e-2;
# bf16 output rounding adds ~2e-3). The returned array is still f32.
OUT_BF16 = int(os.environ.get("K_OUT_BF16", "1"))
OUT_DT = BF16 if OUT_BF16 else F32

# pairs per matmul: 8 -> N = 512 cols = one PSUM bank; 16 -> one 1024-col
# matmul spanning 2 banks (HW-verified correct), halving PE instruction
# count (the PE sequencer is near-saturated dispatching ldweights+matmul).
CHUNK_PAIRS = int(os.environ.get("K_CHUNK_PAIRS", "8"))
MUL_CHUNKS = int(os.environ.get("K_MUL_CHUNKS", str(16 // CHUNK_PAIRS)))
SG_TARGET_PAIRS = int(os.environ.get("K_SG", "96"))
STAGE_BUFS = int(os.environ.get("K_STAGE_BUFS", "4"))
# Parity-split loads: supergroup 2k's W occupies SBUF partitions 0-63 and
# 2k+1's partitions 64-127 (one [128,*] pool tile per sg pair); femT holds
# even-sg fields in the low half, odd-sg fields in the high half. SDMA
# engine k serves a fixed partition set (even engines <64, odd >=64), so
# half-height loads all on partitions 0-63 would bottleneck the 8 even
# engines (~103us); alternating halves splits load traffic evenly.
# matmul requires lhsT/rhs base partitions to match, which this preserves.
PARITY = int(os.environ.get("K_PARITY", "1"))
W_BUFS = int(os.environ.get("K_W_BUFS", "2" if PARITY else "3"))
MM_BUFS = int(os.environ.get("K_MM_BUFS", "4"))
MERGE_FLUSH = int(os.environ.get("K_MERGE_FLUSH", "0"))
FEMT_POOL = int(os.environ.get("K_FEMT_POOL", "0"))

# The elementwise multiply is the compute bottleneck if one engine does it
# all. Each <=16-pair unit is assigned one of three modes:
#   A: DVE multiplies straight from PSUM (f32 operand -> 1x DVE mode)
#   B: Act drains PSUM->bf16 stage; Pool (gpsimd) multiplies in place
#   C: Act drains PSUM->bf16 stage; DVE multiplies in place in bf16,
#      which hits the DVE 2x_1p perf mode (all operands 2-byte packed)
# Weighted round-robin across units balances DVE/Act busy time under the
# HBM floor. B is OFF by default: Pool muls block SWDGE descriptor
# generation on the same Q7, starving the W-load stream (HW-measured:
# any mix with B runs ~15% slower than A+C).
W_A = int(os.environ.get("K_WA", "3"))
W_B = int(os.environ.get("K_WB", "0"))
W_C = int(os.environ.get("K_WC", "8"))
# 1: route the FIRST unit of each flush to B (Pool's slower mul starts
# earliest so the store is tail-gated by the faster DVE muls instead).
B_FIRST = int(os.environ.get("K_B_FIRST", "1"))
# 1: route the LAST unit of each flush to A (single-pass DVE mul has lower
# latency than the Act-copy->DVE-mul chain, so the flush isn't tail-gated).
A_LAST = int(os.environ.get("K_A_LAST", "0"))
FEMB_ENG = os.environ.get("K_FEMB_ENG", "gpsimd")
W_ENG = os.environ.get("K_W_ENG", "gpsimd")
FEMT_ENG = os.environ.get("K_FEMT_ENG", "scalar")
# 1: transposes stage through the matmul PSUM pool instead of a dedicated
# 2-bank psT pool, freeing those banks for a 4th matmul buffer.
PST_SHARE = int(os.environ.get("K_PST_SHARE", "0"))
# 1: host supplies femb pre-transposed [f, d, b] bf16; one DMA loads all
# femT, eliminating the PE-transpose + Act-copy chain entirely.
FEMT_DMA = int(os.environ.get("K_FEMT_DMA", "1"))
# Head-load chunking: fields [0, F0) of femb (rhs operand; sg0 reads right
# fields 1..16) and [0, F0T) of femT (matmul lhsT; early sgs use left
# fields 0..3) load first on the gpsimd queue ahead of W; the remaining
# fields stream on TAIL_ENG's queue so they neither gate the first flush
# nor delay the sg1+ W loads behind them.
F0 = int(os.environ.get("K_F0", "20"))
F0T = int(os.environ.get("K_F0T", "6"))
TAIL_ENG = os.environ.get("K_TAIL_ENG", "scalar")


def _mul_units():
    """Units of <=MUL_CHUNKS*CHUNK_PAIRS pairs within one left-field group.

    Yields (left_field, pair_start, [chunk_cnts], right_field_start)."""
    p0 = 0
    for l in range(NF - 1):
        k = NF - 1 - l
        for u0 in range(0, k, MUL_CHUNKS * CHUNK_PAIRS):
            npair = min(MUL_CHUNKS * CHUNK_PAIRS, k - u0)
            cnts = []
            left = npair
            while left > 0:
                cnts.append(min(CHUNK_PAIRS, left))
                left -= cnts[-1]
            yield (l, p0 + u0, cnts, l + 1 + u0)
        p0 += k


def _sg_size_plan():
    """Supergroup pair-count targets: small head (fast pipeline fill) and
    small tail (fast drain), big middle (DMA efficiency)."""
    hmode = int(os.environ.get("K_HEAD16", "1"))
    head = {0: [32, 64], 1: [16, 32, 64], 2: [8, 16, 32, 64]}[hmode]
    tail = [48, 32, 16]
    mid = PAIRS - sum(head) - sum(tail)
    n_mid = max(1, round(mid / SG_TARGET_PAIRS + 0.5))
    base = mid // n_mid
    plan = head + [base + (1 if i < mid - base * n_mid else 0)
                   for i in range(n_mid)] + tail
    assert sum(plan) == PAIRS
    return plan


def _supergroups():
    """Group consecutive mul-units into flush units per the size plan.

    Each supergroup closes once it REACHES its target (so it may overshoot
    by up to one unit); this avoids a cascade of tiny spill groups."""
    plan = _sg_size_plan()
    sgs, cur, cnt, pi = [], [], 0, 0
    for u in _mul_units():
        cur.append(u)
        cnt += sum(u[2])
        if cnt >= plan[min(pi, len(plan) - 1)]:
            sgs.append(cur)
            cur, cnt = [], 0
            pi += 1
    if cur:
        sgs.append(cur)
    return sgs


def _sg_max_pairs():
    return max(sum(sum(u[2]) for u in sg) for sg in _supergroups())


def _parity_fields():
    """Per-parity femT field lists (first-use order) and field->slot maps.

    A field whose pairs span supergroups of both parities appears in both
    halves (~1.3MB -> ~1.7MB femT stream; the even/odd SDMA balance is
    worth far more than the duplication)."""
    fl, fh = [], []
    for i, sg in enumerate(_supergroups()):
        half = fl if i % 2 == 0 else fh
        for u in sg:
            if u[0] not in half:
                half.append(u[0])
    return (fl, fh,
            {f: i for i, f in enumerate(fl)},
            {f: i for i, f in enumerate(fh)})


def _body(nc, tc, pools, femb, w, out, mode="full"):
    do_load = mode in ("full", "load", "compute")
    do_compute = mode in ("full", "compute")
    do_store = mode in ("full", "store")
    femb_pool, femT_pool, ident_pool, w_pool, psT_pool, psum_pool, stage_pool = pools

    femb, femb_T = femb
    femb_eng = getattr(nc, FEMB_ENG)
    w_eng = getattr(nc, W_ENG)
    femT_eng = getattr(nc, FEMT_ENG)

    if not FEMT_DMA:
        ident = ident_pool.tile([128, 128], FEMB_DT)
        make_identity(nc, ident)

    # Per-b-tile resident tiles: femb rows + transposed fields. The field
    # transposes are emitted just-in-time (per supergroup) so the first
    # flush is not gated on all 80 of them.
    femb_t = []
    femT = []

    def _load_femb(eng, bt, ft, f_lo, f_hi):
        eng.dma_start(
            out=ft[:, f_lo * D:f_hi * D],
            in_=femb[bt * 128:(bt + 1) * 128, f_lo:f_hi].rearrange(
                "b f d -> b (f d)"),
        )

    if PARITY:
        fl, fh, slot_l, slot_h = _parity_fields()
        nl, nh = len(fl), len(fh)
        nmax = max(nl, nh)
        half_n = {0: nl, 1: nh}
        half_off = {0: 0, 1: nl}   # field offset into host femb_T2

    def _load_femT(eng, bt, tt, f_lo, f_hi, half=0):
        # [f, d, b] -> partitions=d, per-field 128-col b slices
        if PARITY:
            off = half_off[half]
            eng.dma_start(
                out=tt[64 * half:64 * half + 64,
                       f_lo * 128:f_hi * 128].rearrange(
                    "d (f b) -> d f b", f=f_hi - f_lo),
                in_=femb_T[off + f_lo:off + f_hi, :,
                           bt * 128:(bt + 1) * 128].rearrange(
                    "f d b -> d f b"),
            )
        else:
            eng.dma_start(
                out=tt[:, f_lo * 128:f_hi * 128].rearrange(
                    "d (f b) -> d f b", f=f_hi - f_lo),
                in_=femb_T[f_lo:f_hi, :, bt * 128:(bt + 1) * 128].rearrange(
                    "f d b -> d f b"),
            )

    f_head = min(F0, NF)
    fT_head = min(F0T, NF)
    for bt in range(B_TILES):
        ft = femb_pool.tile([128, NF * D], FEMB_DT, tag="femb")
        if do_load:
            if FEMB_BF16 == 2:
                raw = femb_pool.tile([128, NF * D], BF16, tag="fraw")
                femb_eng.dma_start(
                    out=raw,
                    in_=femb[bt * 128:(bt + 1) * 128].rearrange(
                        "b f d -> b (f d)"),
                )
                nc.scalar.copy(ft, raw)
            else:
                _load_femb(femb_eng, bt, ft, 0, f_head)
        femb_t.append(ft)
        if not FEMT_POOL:
            if PARITY:
                tt = femT_pool.tile([128, nmax * 128], W_DT, tag="femT")
                if FEMT_DMA and do_load:
                    _load_femT(nc.gpsimd, bt, tt, 0, min(fT_head, nl), 0)
                    _load_femT(nc.gpsimd, bt, tt, 0, min(fT_head, nh), 1)
            else:
                tt = femT_pool.tile([64, NF * 128], W_DT, tag="femT")
                if FEMT_DMA and do_load:
                    _load_femT(nc.gpsimd, bt, tt, 0, fT_head)
            femT.append(tt)

    def _emit_tail_loads():
        if not do_load:
            return
        tail_eng = getattr(nc, TAIL_ENG)
        for bt in range(B_TILES):
            if FEMB_BF16 != 2 and f_head < NF:
                _load_femb(tail_eng, bt, femb_t[bt], f_head, NF)
            if FEMT_DMA and not FEMT_POOL:
                if PARITY:
                    for half in (0, 1):
                        n = half_n[half]
                        if fT_head < n:
                            _load_femT(tail_eng, bt, femT[bt],
                                       fT_head, n, half)
                elif fT_head < NF:
                    _load_femT(tail_eng, bt, femT[bt], fT_head, NF)

    transposed = [dict() for _ in range(B_TILES)]

    def ensure_fields(bt, fields):
        if FEMT_DMA:
            return
        for f in fields:
            if f in transposed[bt]:
                continue
            ps = (psum_pool if PST_SHARE else psT_pool).tile(
                [64, 128], FEMB_DT, tag="mm" if PST_SHARE else None)
            nc.tensor.transpose(ps, femb_t[bt][:, f * D:(f + 1) * D], ident)

            def _cp(dst, src):
                if FEMT_ENG == "vector":
                    nc.vector.tensor_copy(dst, src)
                else:
                    femT_eng.copy(dst, src)

            if FEMT_POOL:
                ftile = femT_pool.tile([64, 128], W_DT, tag="femT")
                _cp(ftile, ps)
                transposed[bt][f] = ftile
            else:
                _cp(femT[bt][:, f * 128:(f + 1) * 128], ps)
                transposed[bt][f] = True

    def lhsT_for(bt, f, par=0):
        if FEMT_POOL:
            return transposed[bt][f]
        if PARITY:
            s = slot_l[f] if par == 0 else slot_h[f]
            return femT[bt][64 * par:64 * par + 64,
                            s * 128:(s + 1) * 128]
        return femT[bt][:, f * 128:(f + 1) * 128]

    # Smooth weighted round-robin over unit modes; forced-B units (first of
    # each flush) bill against B's share so the global ratio stays on target.
    wsum = {"A": W_A, "B": W_B, "C": W_C}
    credit = {m: 0.0 for m in "ABC"}
    tot_w = sum(wsum.values())

    def pick_mode(force_b, force_a=False):
        for m in "ABC":
            credit[m] += wsum[m]
        if force_a and W_A > 0:
            m = "A"
        elif force_b and W_B > 0:
            m = "B"
        else:
            m = max("ABC", key=lambda k: (credit[k], k))
        credit[m] -= tot_w
        return m

    first_sg = True
    wpair = None
    for si, sg in enumerate(_supergroups()):
        sg_p0 = sg[0][1]
        sg_np = sum(sum(u[2]) for u in sg)
        # One DMA loads this supergroup's W; both b-tiles reuse it. In
        # parity mode consecutive sgs share a [128,*] tile (low/high half).
        par = si % 2 if PARITY else 0
        if PARITY:
            if par == 0:
                wpair = w_pool.tile([128, _sg_max_pairs(), D], W_DT, tag="w")
            wsg = wpair[64 * par:64 * par + 64]
        else:
            wsg = w_pool.tile([64, _sg_max_pairs(), D], W_DT, tag="w")
        if do_load:
            w_eng.dma_start(
                out=wsg[:, :sg_np, :],
                in_=w[:, sg_p0:sg_p0 + sg_np, :] if W_BF16
                else w[:, sg_p0:sg_p0 + sg_np, :].bitcast(F32R),
            )
        if first_sg:
            _emit_tail_loads()
            first_sg = False
        if MERGE_FLUSH:
            mstage = stage_pool.tile([128, B_TILES, _sg_max_pairs() * D],
                                     OUT_DT, tag="stage")
        for bt in range(B_TILES):
            if do_compute:
                ensure_fields(bt, sorted({u[0] for u in sg}))
            if MERGE_FLUSH:
                stage = mstage[:, bt, :]
            else:
                stage = stage_pool.tile([128, _sg_max_pairs() * D], OUT_DT,
                                        tag="stage")
            if do_store and not do_compute:
                # touch the tile so Tile allocates it; garbage data is fine
                nc.vector.memset(stage[:, :8], 0.0)
            if do_compute:
                for ui, (l, p0, cnts, r0) in enumerate(sg):
                    un = sum(cnts)
                    ps = psum_pool.tile([128, MUL_CHUNKS * CHUNK_PAIRS * D], F32,
                                        tag="mm")
                    off = 0
                    for cnt in cnts:
                        woff = p0 - sg_p0 + off
                        nc.tensor.matmul(
                            ps[:, off * D:(off + cnt) * D],
                            lhsT=lhsT_for(bt, l, par),
                            rhs=wsg[:, woff:woff + cnt, :].rearrange("d p e -> d (p e)"),
                            start=True,
                            stop=True,
                        )
                        off += cnt
                    co = (p0 - sg_p0) * D
                    st = stage[:, co:co + un * D]
                    rhs_fe = femb_t[bt][:, r0 * D:(r0 + un) * D]
                    m = pick_mode(B_FIRST and ui == 0,
                                  A_LAST and ui == len(sg) - 1)
                    if m == "A":
                        nc.vector.tensor_mul(st, ps[:, :un * D], rhs_fe)
                    elif m == "B":
                        # Act drains PSUM->stage; Pool multiplies in place.
                        nc.scalar.copy(st, ps[:, :un * D])
                        nc.gpsimd.tensor_mul(st, st, rhs_fe)
                    else:
                        # Act drains PSUM->stage; DVE 2x bf16 mul in place.
                        nc.scalar.copy(st, ps[:, :un * D])
                        nc.vector.tensor_mul(st, st, rhs_fe)
            if do_store and not MERGE_FLUSH:
                nc.sync.dma_start(
                    out=out[bt * 128:(bt + 1) * 128, sg_p0:sg_p0 + sg_np].rearrange(
                        "b p e -> b (p e)"
                    ),
                    in_=stage[:, :sg_np * D],
                )
        if do_store and MERGE_FLUSH:
            nc.sync.dma_start(
                out=out[:, sg_p0:sg_p0 + sg_np].rearrange(
                    "(t b) p e -> b t (p e)", b=128
                ),
                in_=mstage[:, :, :sg_np * D],
            )


def build_kernel(reps: int = 1, mode: str = "full") -> bass.Bass:
    """Build + finalize the per-core Bass module.

    reps > 1 wraps the whole body in a hardware loop (for timing runs)."""
    nc = bacc.Bacc("TRN2", target_bir_lowering=False)
    femb = nc.dram_tensor("feature_emb", [B_LOCAL, NF, D], FEMB_DRAM_DT,
                          kind="ExternalInput")
    if PARITY:
        fl, fh, _, _ = _parity_fields()
        nft = len(fl) + len(fh)
    else:
        nft = NF
    femb_T = (nc.dram_tensor("femb_T", [nft, D, B_LOCAL], BF16,
                             kind="ExternalInput") if FEMT_DMA else None)
    # host-swizzled to [d, p, e] for contiguous per-partition DMA lines
    w = nc.dram_tensor("bilinear_W", [D, PAIRS, D], BF16 if W_BF16 else F32,
                       kind="ExternalInput")
    out = nc.dram_tensor("out", [B_LOCAL, PAIRS, D], OUT_DT,
                         kind="ExternalOutput")

    with tile.TileContext(nc) as tc:
        from contextlib import ExitStack
        with ExitStack() as _stack:
            femb_pool = _stack.enter_context(
                tc.tile_pool(name="femb", bufs=B_TILES))
            femT_pool = _stack.enter_context(
                tc.tile_pool(name="femT", bufs=(12 if FEMT_POOL else B_TILES)))
            ident_pool = _stack.enter_context(tc.tile_pool(name="ident", bufs=1))
            w_pool = _stack.enter_context(tc.tile_pool(name="w", bufs=W_BUFS))
            psum_pool = _stack.enter_context(
                tc.tile_pool(name="mm", bufs=MM_BUFS, space="PSUM"))
            psT_pool = psum_pool if PST_SHARE else _stack.enter_context(
                tc.tile_pool(name="psT", bufs=2, space="PSUM"))
            stage_pool = _stack.enter_context(
                tc.tile_pool(name="stage", bufs=STAGE_BUFS))
            pools = (femb_pool, femT_pool, ident_pool, w_pool, psT_pool,
                     psum_pool, stage_pool)
            if reps == 1:
                _body(nc, tc, pools, (femb, femb_T), w, out, mode)
            else:
                with tc.For_i(0, reps, 1):
                    _body(nc, tc, pools, (femb, femb_T), w, out, mode)
    nc.finalize()
    return nc


_CACHED_NC = None


def make_in_maps(feature_emb: np.ndarray, bilinear_W: np.ndarray):
    feature_emb = np.ascontiguousarray(np.asarray(feature_emb, dtype=np.float32))
    if FEMB_BF16:
        import ml_dtypes
        feature_emb = feature_emb.astype(ml_dtypes.bfloat16)
    bilinear_W = np.asarray(bilinear_W, dtype=np.float32)
    assert feature_emb.shape == (BATCH, NF, D)
    assert bilinear_W.shape == (PAIRS, D, D)
    w_swz = np.ascontiguousarray(bilinear_W.transpose(1, 0, 2))  # [d, p, e]
    if W_BF16:
        import ml_dtypes
        w_swz = w_swz.astype(ml_dtypes.bfloat16)
    maps = []
    for c in range(N_CORES):
        m = {
            "feature_emb": feature_emb[c * B_LOCAL:(c + 1) * B_LOCAL],
            "bilinear_W": w_swz,
        }
        if FEMT_DMA:
            import ml_dtypes
            fT = np.asarray(
                m["feature_emb"], dtype=np.float32).transpose(1, 2, 0)
            if PARITY:
                fl, fh, _, _ = _parity_fields()
                fT = fT[fl + fh]
            m["femb_T"] = np.ascontiguousarray(fT.astype(ml_dtypes.bfloat16))
        maps.append(m)
    return maps


def kernel(feature_emb: np.ndarray, bilinear_W: np.ndarray,
           left_idx: np.ndarray = None, right_idx: np.ndarray = None,
           **_ignored) -> np.ndarray:
    global _CACHED_NC
    if _CACHED_NC is None:
        _CACHED_NC = build_kernel(reps=1)
    nc = _CACHED_NC

    in_maps = make_in_maps(feature_emb, bilinear_W)
    # The NRT occasionally reports a transient "exec unit unrecoverable" if a
    # previous process wedged a core; it clears on retry.
    last_err = None
    for attempt in range(3):
        try:
            res = run_bass_kernel_spmd(nc, in_maps, list(range(N_CORES)))
            break
        except Exception as e:  # noqa: BLE001
            last_err = e
            time.sleep(5.0)
    else:
        raise last_err
    full = np.concatenate([res.results[c]["out"] for c in range(N_CORES)],
                          axis=0)
    return np.ascontiguousarray(full.astype(np.float32))



# revision 30
# speedup vs baseline: 1.2515x; 1.0447x over previous
"""Trainium2 Bass kernel for BilinearInteractionV2.

out[b, p, e] = (sum_d femb[b, left[p], d] * W[p, d, e]) * femb[b, right[p], e]

feature_emb: [2048, 40, 64] f32, bilinear_W: [780, 64, 64] f32,
left/right idx = upper-triangle pairs in combinations order (left-major).

Sharding: batch split across 8 NeuronCores (pure data parallel), W replicated.

Per-core plan (B_local = 256 = 2 b-tiles of 128 rows), HW-measured:
everything streams bf16 (output upcast to f32 on the host; the 2e-2
rel-err budget absorbs ~3.4e-3 median rounding error), so the HBM
traffic is ~25.6MB stores + ~9.4MB loads ~= 97us at the ~368GB/s
HBM/NC rate, with the drain/multiply chain (~98us) right at that floor:
  - Output stores get the sync HWDGE ring to themselves (HWDGE rings are
    FIFO per engine, so anything sharing it would serialize with 25.6MB
    of stores); W/femb/femT head loads go through the gpsimd SWDGE ring,
    femb/femT tails through the Act HWDGE ring so they don't delay the
    per-supergroup W stream behind them.
  - W host-swizzled to [d, p, e] bf16 (halves the read, contiguous DMA
    lines); femT (lhsT operand, d on partitions) host-transposed and
    DMA'd directly. SDMA engine k serves a fixed partition set (even
    engines partitions <64, odd >=64), so the 64-tall W/femT tiles are
    parity-split: supergroup 2k's W in partitions 0-63, 2k+1's in
    64-127 (one [128,*] tile per sg pair), and femT fields placed by the
    parity of the sgs that use them (boundary fields duplicated). This
    keeps both matmul operands at one base partition (HW requirement)
    while spreading load bytes over all 16 SDMA engines.
  - Pairs grouped by left field (combinations order is left-major); per
    unit of <=16 pairs one stationary femT slice serves 2 bf16 matmuls
    [64,128]^T @ [64, 8*64] -> PSUM [128, 16*64] (walrus rejects 1024-col
    matmul outputs, so 2 per unit; 4-deep PSUM pool).
  - The elementwise *right multiply is split across engines per unit:
    A-units let the DVE multiply straight from PSUM (f32, 1x mode);
    C-units have Act drain PSUM->bf16 stage and the DVE multiply in
    place in bf16, hitting the DVE 2x_1p mode (all operands 2-byte).
    ~27% A / 73% C balances DVE vs Act at ~75us busy each. B-units
    (Act drain + Pool mul) are disabled: Pool muls block SWDGE
    descriptor generation on the same Q7 and starve the load stream
    (HW-measured ~15% regression for any B mix).
  - Stage tiles flush to DRAM in ~1.5MB bf16 stores per supergroup (~96
    pairs, smaller head groups for fast fill, small tail for drain);
    femb/femT head chunks cover the first supergroups so the first
    flush isn't gated on the full load stream.
"""

import os
import time

import numpy as np

import concourse.bass as bass
import concourse.mybir as mybir
import concourse.tile as tile
from concourse import bacc
from concourse.bass_utils import run_bass_kernel_spmd
from concourse.masks import make_identity

N_CORES = 8
BATCH = 2048
B_LOCAL = BATCH // N_CORES  # 256
B_TILES = B_LOCAL // 128    # 2
NF = 40
D = 64
PAIRS = NF * (NF - 1) // 2  # 780
F32 = mybir.dt.float32
F32R = mybir.dt.float32r
BF16 = mybir.dt.bfloat16

# W (and the matmul lhsT) in bf16: halves the dominant replicated-W HBM
# read (12.8MB -> 6.4MB per core). Precision: bf16 product terms over a
# 64-term contraction give ~2e-3 relative error, far inside the 2e-2 gate.
W_BF16 = int(os.environ.get("K_W_BF16", "1"))
W_DT = BF16 if W_BF16 else F32R
# femb bf16: 1 = bf16 end-to-end — halves the femb HBM read AND makes the
# C-mode (Act-drain + DVE 2x bf16 mul) stage multiply all-bf16, unlocking
# the DVE 2x_1p perf mode; 2 = load bf16 and upcast once to f32 on Act.
FEMB_BF16 = int(os.environ.get("K_FEMB_BF16", "1"))
FEMB_DT = BF16 if FEMB_BF16 == 1 else F32
FEMB_DRAM_DT = BF16 if FEMB_BF16 else F32
# Store the output as bf16 and upcast to f32 on the host: halves the
# dominant 51.1MB/core store stream (the harness gates on rel err 2e-2;
# bf16 output rounding adds ~2e-3). The returned array is still f32.
OUT_BF16 = int(os.environ.get("K_OUT_BF16", "1"))
OUT_DT = BF16 if OUT_BF16 else F32

# pairs per matmul: 8 -> N = 512 cols = one PSUM bank; 16 -> one 1024-col
# matmul spanning 2 banks (HW-verified correct), halving PE instruction
# count (the PE sequencer is near-saturated dispatching ldweights+matmul).
CHUNK_PAIRS = int(os.environ.get("K_CHUNK_PAIRS", "8"))
MUL_CHUNKS = int(os.environ.get("K_MUL_CHUNKS", str(16 // CHUNK_PAIRS)))
SG_TARGET_PAIRS = int(os.environ.get("K_SG", "96"))
STAGE_BUFS = int(os.environ.get("K_STAGE_BUFS", "4"))
# Parity-split loads: supergroup 2k's W occupies SBUF partitions 0-63 and
# 2k+1's partitions 64-127 (one [128,*] pool tile per sg pair); femT holds
# even-sg fields in the low half, odd-sg fields in the high half. SDMA
# engine k serves a fixed partition set (even engines <64, odd >=64), so
# half-height loads all on partitions 0-63 would bottleneck the 8 even
# engines (~103us); alternating halves splits load traffic evenly.
# matmul requires lhsT/rhs base partitions to match, which this preserves.
PARITY = int(os.environ.get("K_PARITY", "1"))
W_BUFS = int(os.environ.get("K_W_BUFS", "2" if PARITY else "3"))
MM_BUFS = int(os.environ.get("K_MM_BUFS", "4"))
MERGE_FLUSH = int(os.environ.get("K_MERGE_FLUSH", "0"))
FEMT_POOL = int(os.environ.get("K_FEMT_POOL", "0"))

# The elementwise multiply is the compute bottleneck if one engine does it
# all. Each <=16-pair unit is assigned one of three modes:
#   A: DVE multiplies straight from PSUM (f32 operand -> 1x DVE mode)
#   B: Act drains PSUM->bf16 stage; Pool (gpsimd) multiplies in place
#   C: Act drains PSUM->bf16 stage; DVE multiplies in place in bf16,
#      which hits the DVE 2x_1p perf mode (all operands 2-byte packed)
# Weighted round-robin across units balances DVE/Act busy time under the
# HBM floor. B is OFF by default: Pool muls block SWDGE descriptor
# generation on the same Q7, starving the W-load stream (HW-measured:
# any mix with B runs ~15% slower than A+C).
W_A = int(os.environ.get("K_WA", "3"))
W_B = int(os.environ.get("K_WB", "0"))
W_C = int(os.environ.get("K_WC", "8"))
# 1: route the FIRST unit of each flush to B (Pool's slower mul starts
# earliest so the store is tail-gated by the faster DVE muls instead).
B_FIRST = int(os.environ.get("K_B_FIRST", "1"))
# 1: route the LAST unit of each flush to A (single-pass DVE mul has lower
# latency than the Act-copy->DVE-mul chain, so the flush isn't tail-gated).
A_LAST = int(os.environ.get("K_A_LAST", "1"))
# From sg index B_FROM_SG onward, switch to weights WA2/WB2/WC2: Pool muls
# only poison SWDGE while load descriptors are still being generated, so
# B-units are safe (and free capacity) once the last W DMA is emitted.
B_FROM_SG = int(os.environ.get("K_B_FROM_SG", "99"))
W_A2 = int(os.environ.get("K_WA2", "2"))
W_B2 = int(os.environ.get("K_WB2", "3"))
W_C2 = int(os.environ.get("K_WC2", "6"))
FEMB_ENG = os.environ.get("K_FEMB_ENG", "gpsimd")
W_ENG = os.environ.get("K_W_ENG", "gpsimd")
FEMT_ENG = os.environ.get("K_FEMT_ENG", "scalar")
# 1: transposes stage through the matmul PSUM pool instead of a dedicated
# 2-bank psT pool, freeing those banks for a 4th matmul buffer.
PST_SHARE = int(os.environ.get("K_PST_SHARE", "0"))
# 1: host supplies femb pre-transposed [f, d, b] bf16; one DMA loads all
# femT, eliminating the PE-transpose + Act-copy chain entirely.
FEMT_DMA = int(os.environ.get("K_FEMT_DMA", "1"))
# Head-load chunking: fields [0, F0) of femb (rhs operand; sg0 reads right
# fields 1..16) and [0, F0T) of femT (matmul lhsT; early sgs use left
# fields 0..3) load first on the gpsimd queue ahead of W; the remaining
# fields stream on TAIL_ENG's queue so they neither gate the first flush
# nor delay the sg1+ W loads behind them.
F0 = int(os.environ.get("K_F0", "20"))
F0T = int(os.environ.get("K_F0T", "4"))
TAIL_ENG = os.environ.get("K_TAIL_ENG", "scalar")


def _mul_units():
    """Units of <=MUL_CHUNKS*CHUNK_PAIRS pairs within one left-field group.

    Yields (left_field, pair_start, [chunk_cnts], right_field_start)."""
    p0 = 0
    for l in range(NF - 1):
        k = NF - 1 - l
        for u0 in range(0, k, MUL_CHUNKS * CHUNK_PAIRS):
            npair = min(MUL_CHUNKS * CHUNK_PAIRS, k - u0)
            cnts = []
            left = npair
            while left > 0:
                cnts.append(min(CHUNK_PAIRS, left))
                left -= cnts[-1]
            yield (l, p0 + u0, cnts, l + 1 + u0)
        p0 += k


def _sg_size_plan():
    """Supergroup pair-count targets: small head (fast pipeline fill) and
    small tail (fast drain), big middle (DMA efficiency)."""
    hmode = int(os.environ.get("K_HEAD16", "1"))
    head = {0: [32, 64], 1: [16, 32, 64], 2: [8, 16, 32, 64]}[hmode]
    tail = [48, 32, 16]
    mid = PAIRS - sum(head) - sum(tail)
    n_mid = max(1, round(mid / SG_TARGET_PAIRS + 0.5))
    base = mid // n_mid
    plan = head + [base + (1 if i < mid - base * n_mid else 0)
                   for i in range(n_mid)] + tail
    assert sum(plan) == PAIRS
    return plan


def _supergroups():
    """Group consecutive mul-units into flush units per the size plan.

    Each supergroup closes once it REACHES its target (so it may overshoot
    by up to one unit); this avoids a cascade of tiny spill groups."""
    plan = _sg_size_plan()
    sgs, cur, cnt, pi = [], [], 0, 0
    for u in _mul_units():
        cur.append(u)
        cnt += sum(u[2])
        if cnt >= plan[min(pi, len(plan) - 1)]:
            sgs.append(cur)
            cur, cnt = [], 0
            pi += 1
    if cur:
        sgs.append(cur)
    return sgs


def _sg_max_pairs():
    return max(sum(sum(u[2]) for u in sg) for sg in _supergroups())


def _parity_fields():
    """Per-parity femT field lists (first-use order) and field->slot maps.

    A field whose pairs span supergroups of both parities appears in both
    halves (~1.3MB -> ~1.7MB femT stream; the even/odd SDMA balance is
    worth far more than the duplication)."""
    fl, fh = [], []
    for i, sg in enumerate(_supergroups()):
        half = fl if i % 2 == 0 else fh
        for u in sg:
            if u[0] not in half:
                half.append(u[0])
    return (fl, fh,
            {f: i for i, f in enumerate(fl)},
            {f: i for i, f in enumerate(fh)})


def _body(nc, tc, pools, femb, w, out, mode="full"):
    do_load = mode in ("full", "load", "compute")
    do_compute = mode in ("full", "compute")
    do_store = mode in ("full", "store")
    femb_pool, femT_pool, ident_pool, w_pool, psT_pool, psum_pool, stage_pool = pools

    femb, femb_T = femb
    femb_eng = getattr(nc, FEMB_ENG)
    w_eng = getattr(nc, W_ENG)
    femT_eng = getattr(nc, FEMT_ENG)

    if not FEMT_DMA:
        ident = ident_pool.tile([128, 128], FEMB_DT)
        make_identity(nc, ident)

    # Per-b-tile resident tiles: femb rows + transposed fields. The field
    # transposes are emitted just-in-time (per supergroup) so the first
    # flush is not gated on all 80 of them.
    femb_t = []
    femT = []

    def _load_femb(eng, bt, ft, f_lo, f_hi):
        eng.dma_start(
            out=ft[:, f_lo * D:f_hi * D],
            in_=femb[bt * 128:(bt + 1) * 128, f_lo:f_hi].rearrange(
                "b f d -> b (f d)"),
        )

    if PARITY:
        fl, fh, slot_l, slot_h = _parity_fields()
        nl, nh = len(fl), len(fh)
        nmax = max(nl, nh)
        half_n = {0: nl, 1: nh}
        half_off = {0: 0, 1: nl}   # field offset into host femb_T2

    def _load_femT(eng, bt, tt, f_lo, f_hi, half=0):
        # [f, d, b] -> partitions=d, per-field 128-col b slices
        if PARITY:
            off = half_off[half]
            eng.dma_start(
                out=tt[64 * half:64 * half + 64,
                       f_lo * 128:f_hi * 128].rearrange(
                    "d (f b) -> d f b", f=f_hi - f_lo),
                in_=femb_T[off + f_lo:off + f_hi, :,
                           bt * 128:(bt + 1) * 128].rearrange(
                    "f d b -> d f b"),
            )
        else:
            eng.dma_start(
                out=tt[:, f_lo * 128:f_hi * 128].rearrange(
                    "d (f b) -> d f b", f=f_hi - f_lo),
                in_=femb_T[f_lo:f_hi, :, bt * 128:(bt + 1) * 128].rearrange(
                    "f d b -> d f b"),
            )

    f_head = min(F0, NF)
    fT_head = min(F0T, NF)
    for bt in range(B_TILES):
        ft = femb_pool.tile([128, NF * D], FEMB_DT, tag="femb")
        if do_load:
            if FEMB_BF16 == 2:
                raw = femb_pool.tile([128, NF * D], BF16, tag="fraw")
                femb_eng.dma_start(
                    out=raw,
                    in_=femb[bt * 128:(bt + 1) * 128].rearrange(
                        "b f d -> b (f d)"),
                )
                nc.scalar.copy(ft, raw)
            else:
                _load_femb(femb_eng, bt, ft, 0, f_head)
        femb_t.append(ft)
        if not FEMT_POOL:
            if PARITY:
                tt = femT_pool.tile([128, nmax * 128], W_DT, tag="femT")
                if FEMT_DMA and do_load:
                    _load_femT(nc.gpsimd, bt, tt, 0, min(fT_head, nl), 0)
                    _load_femT(nc.gpsimd, bt, tt, 0, min(fT_head, nh), 1)
            else:
                tt = femT_pool.tile([64, NF * 128], W_DT, tag="femT")
                if FEMT_DMA and do_load:
                    _load_femT(nc.gpsimd, bt, tt, 0, fT_head)
            femT.append(tt)

    def _emit_tail_loads():
        if not do_load:
            return
        tail_eng = getattr(nc, TAIL_ENG)
        for bt in range(B_TILES):
            if FEMB_BF16 != 2 and f_head < NF:
                _load_femb(tail_eng, bt, femb_t[bt], f_head, NF)
            if FEMT_DMA and not FEMT_POOL:
                if PARITY:
                    for half in (0, 1):
                        n = half_n[half]
                        if fT_head < n:
                            _load_femT(tail_eng, bt, femT[bt],
                                       fT_head, n, half)
                elif fT_head < NF:
                    _load_femT(tail_eng, bt, femT[bt], fT_head, NF)

    transposed = [dict() for _ in range(B_TILES)]

    def ensure_fields(bt, fields):
        if FEMT_DMA:
            return
        for f in fields:
            if f in transposed[bt]:
                continue
            ps = (psum_pool if PST_SHARE else psT_pool).tile(
                [64, 128], FEMB_DT, tag="mm" if PST_SHARE else None)
            nc.tensor.transpose(ps, femb_t[bt][:, f * D:(f + 1) * D], ident)

            def _cp(dst, src):
                if FEMT_ENG == "vector":
                    nc.vector.tensor_copy(dst, src)
                else:
                    femT_eng.copy(dst, src)

            if FEMT_POOL:
                ftile = femT_pool.tile([64, 128], W_DT, tag="femT")
                _cp(ftile, ps)
                transposed[bt][f] = ftile
            else:
                _cp(femT[bt][:, f * 128:(f + 1) * 128], ps)
                transposed[bt][f] = True

    def lhsT_for(bt, f, par=0):
        if FEMT_POOL:
            return transposed[bt][f]
        if PARITY:
            s = slot_l[f] if par == 0 else slot_h[f]
            return femT[bt][64 * par:64 * par + 64,
                            s * 128:(s + 1) * 128]
        return femT[bt][:, f * 128:(f + 1) * 128]

    # Smooth weighted round-robin over unit modes; forced-B units (first of
    # each flush) bill against B's share so the global ratio stays on target.
    wsum = dict(zip("ABC", (W_A, W_B, W_C)))
    wsum2 = dict(zip("ABC", (W_A2, W_B2, W_C2)))
    credit = {m: 0.0 for m in "ABC"}

    def pick_mode(force_b, force_a=False, late=False):
        w = wsum2 if late else wsum
        tot_w = sum(w.values())
        for m in "ABC":
            credit[m] += w[m]
        if force_a and w["A"] > 0:
            m = "A"
        elif force_b and w["B"] > 0:
            m = "B"
        else:
            m = max("ABC", key=lambda k: (credit[k], k))
        credit[m] -= tot_w
        return m

    first_sg = True
    sgs = _supergroups()
    # In parity mode consecutive sgs share one [128,*] W tile (low/high
    # half); allocate per sg-pair so the Tile scope tracker sees a clean
    # alloc/release per tag cycle.
    wpairs = {}
    for si, sg in enumerate(sgs):
        sg_p0 = sg[0][1]
        sg_np = sum(sum(u[2]) for u in sg)
        # One DMA loads this supergroup's W; both b-tiles reuse it.
        par = si % 2 if PARITY else 0
        if PARITY:
            if par == 0:
                wpairs[si // 2] = w_pool.tile([128, _sg_max_pairs(), D],
                                              W_DT, tag="w",
                                              name=f"wpair{si // 2}")
            wsg = wpairs[si // 2][64 * par:64 * par + 64]
        else:
            wsg = w_pool.tile([64, _sg_max_pairs(), D], W_DT, tag="w")
        if do_load:
            w_eng.dma_start(
                out=wsg[:, :sg_np, :],
                in_=w[:, sg_p0:sg_p0 + sg_np, :] if W_BF16
                else w[:, sg_p0:sg_p0 + sg_np, :].bitcast(F32R),
            )
        if first_sg:
            _emit_tail_loads()
            first_sg = False
        if MERGE_FLUSH:
            mstage = stage_pool.tile([128, B_TILES, _sg_max_pairs() * D],
                                     OUT_DT, tag="stage")
        for bt in range(B_TILES):
            if do_compute:
                ensure_fields(bt, sorted({u[0] for u in sg}))
            if MERGE_FLUSH:
                stage = mstage[:, bt, :]
            else:
                stage = stage_pool.tile([128, _sg_max_pairs() * D], OUT_DT,
                                        tag="stage")
            if do_store and not do_compute:
                # touch the tile so Tile allocates it; garbage data is fine
                nc.vector.memset(stage[:, :8], 0.0)
            if do_compute:
                for ui, (l, p0, cnts, r0) in enumerate(sg):
                    un = sum(cnts)
                    ps = psum_pool.tile([128, MUL_CHUNKS * CHUNK_PAIRS * D], F32,
                                        tag="mm")
                    off = 0
                    for cnt in cnts:
                        woff = p0 - sg_p0 + off
                        nc.tensor.matmul(
                            ps[:, off * D:(off + cnt) * D],
                            lhsT=lhsT_for(bt, l, par),
                            rhs=wsg[:, woff:woff + cnt, :].rearrange("d p e -> d (p e)"),
                            start=True,
                            stop=True,
                        )
                        off += cnt
                    co = (p0 - sg_p0) * D
                    st = stage[:, co:co + un * D]
                    rhs_fe = femb_t[bt][:, r0 * D:(r0 + un) * D]
                    m = pick_mode(B_FIRST and ui == 0,
                                  A_LAST and ui == len(sg) - 1,
                                  si >= B_FROM_SG)
                    if m == "A":
                        nc.vector.tensor_mul(st, ps[:, :un * D], rhs_fe)
                    elif m == "B":
                        # Act drains PSUM->stage; Pool multiplies in place.
                        nc.scalar.copy(st, ps[:, :un * D])
                        nc.gpsimd.tensor_mul(st, st, rhs_fe)
                    else:
                        # Act drains PSUM->stage; DVE 2x bf16 mul in place.
                        nc.scalar.copy(st, ps[:, :un * D])
                        nc.vector.tensor_mul(st, st, rhs_fe)
            if do_store and not MERGE_FLUSH:
                nc.sync.dma_start(
                    out=out[bt * 128:(bt + 1) * 128, sg_p0:sg_p0 + sg_np].rearrange(
                        "b p e -> b (p e)"
                    ),
                    in_=stage[:, :sg_np * D],
                )
        if do_store and MERGE_FLUSH:
            nc.sync.dma_start(
                out=out[:, sg_p0:sg_p0 + sg_np].rearrange(
                    "(t b) p e -> b t (p e)", b=128
                ),
                in_=mstage[:, :, :sg_np * D],
            )


def build_kernel(reps: int = 1, mode: str = "full") -> bass.Bass:
    """Build + finalize the per-core Bass module.

    reps > 1 wraps the whole body in a hardware loop (for timing runs)."""
    nc = bacc.Bacc("TRN2", target_bir_lowering=False)
    femb = nc.dram_tensor("feature_emb", [B_LOCAL, NF, D], FEMB_DRAM_DT,
                          kind="ExternalInput")
    if PARITY:
        fl, fh, _, _ = _parity_fields()
        nft = len(fl) + len(fh)
    else:
        nft = NF
    femb_T = (nc.dram_tensor("femb_T", [nft, D, B_LOCAL], BF16,
                             kind="ExternalInput") if FEMT_DMA else None)
    # host-swizzled to [d, p, e] for contiguous per-partition DMA lines
    w = nc.dram_tensor("bilinear_W", [D, PAIRS, D], BF16 if W_BF16 else F32,
                       kind="ExternalInput")
    out = nc.dram_tensor("out", [B_LOCAL, PAIRS, D], OUT_DT,
                         kind="ExternalOutput")

    with tile.TileContext(nc) as tc:
        from contextlib import ExitStack
        with ExitStack() as _stack:
            femb_pool = _stack.enter_context(
                tc.tile_pool(name="femb", bufs=B_TILES))
            femT_pool = _stack.enter_context(
                tc.tile_pool(name="femT", bufs=(12 if FEMT_POOL else B_TILES)))
            ident_pool = _stack.enter_context(tc.tile_pool(name="ident", bufs=1))
            w_pool = _stack.enter_context(tc.tile_pool(name="w", bufs=W_BUFS))
            psum_pool = _stack.enter_context(
                tc.tile_pool(name="mm", bufs=MM_BUFS, space="PSUM"))
            psT_pool = psum_pool if PST_SHARE else _stack.enter_context(
                tc.tile_pool(name="psT", bufs=2, space="PSUM"))
            stage_pool = _stack.enter_context(
                tc.tile_pool(name="stage", bufs=STAGE_BUFS))
            pools = (femb_pool, femT_pool, ident_pool, w_pool, psT_pool,
                     psum_pool, stage_pool)
            if reps == 1:
                _body(nc, tc, pools, (femb, femb_T), w, out, mode)
            else:
                with tc.For_i(0, reps, 1):
                    _body(nc, tc, pools, (femb, femb_T), w, out, mode)
    nc.finalize()
    return nc


_CACHED_NC = None


def make_in_maps(feature_emb: np.ndarray, bilinear_W: np.ndarray):
    feature_emb = np.ascontiguousarray(np.asarray(feature_emb, dtype=np.float32))
    if FEMB_BF16:
        import ml_dtypes
        feature_emb = feature_emb.astype(ml_dtypes.bfloat16)
    bilinear_W = np.asarray(bilinear_W, dtype=np.float32)
    assert feature_emb.shape == (BATCH, NF, D)
    assert bilinear_W.shape == (PAIRS, D, D)
    w_swz = np.ascontiguousarray(bilinear_W.transpose(1, 0, 2))  # [d, p, e]
    if W_BF16:
        import ml_dtypes
        w_swz = w_swz.astype(ml_dtypes.bfloat16)
    maps = []
    for c in range(N_CORES):
        m = {
            "feature_emb": feature_emb[c * B_LOCAL:(c + 1) * B_LOCAL],
            "bilinear_W": w_swz,
        }
        if FEMT_DMA:
            import ml_dtypes
            fT = np.asarray(
                m["feature_emb"], dtype=np.float32).transpose(1, 2, 0)
            if PARITY:
                fl, fh, _, _ = _parity_fields()
                fT = fT[fl + fh]
            m["femb_T"] = np.ascontiguousarray(fT.astype(ml_dtypes.bfloat16))
        maps.append(m)
    return maps


def kernel(feature_emb: np.ndarray, bilinear_W: np.ndarray,
           left_idx: np.ndarray = None, right_idx: np.ndarray = None,
           **_ignored) -> np.ndarray:
    global _CACHED_NC
    if _CACHED_NC is None:
        _CACHED_NC = build_kernel(reps=1)
    nc = _CACHED_NC

    in_maps = make_in_maps(feature_emb, bilinear_W)
    # The NRT occasionally reports a transient "exec unit unrecoverable" if a
    # previous process wedged a core; it clears on retry.
    last_err = None
    for attempt in range(3):
        try:
            res = run_bass_kernel_spmd(nc, in_maps, list(range(N_CORES)))
            break
        except Exception as e:  # noqa: BLE001
            last_err = e
            time.sleep(5.0)
    else:
        raise last_err
    full = np.concatenate([res.results[c]["out"] for c in range(N_CORES)],
                          axis=0)
    return np.ascontiguousarray(full.astype(np.float32))

